# revision 40
# baseline (speedup 1.0000x reference)
"""GAT edge->relation aggregation on 8 trn2 NeuronCores.

Strategy (node-sharded, matmul-centric):
  out_tau[r] = (1/z_tau[r]) * sum_e w_tau_e * feat_tau[node_e],  tau in {1,2}
with w_tau_e = exp(leakyrelu(s_a[own_e] + s_b[rem_e])).  Softmax max-subtraction
cancels algebraically (z >> EPS), so it is skipped.

Per core (owns 6250 nodes, padded to 6272 = 49*128):
  1. One matmul pass over x_shard gives the four per-node scalar scores
     (weights folded host-side: W_h@a_h1 etc) -> AllGather of the [6272,4]
     score table; a second pass (same resident xT tiles) gives the feature
     projections [x_r_h | x_r_t].
  2. Edge instances are sharded by owning node (h for term1, t for term2).
     Remote-side scores are delivered via: run-start local_scatter from the
     gathered table into a compact per-partition grid; log-doubling fill
     (DVE) expands runs; two local_scatters group by dest partition; one
     SBUF->SBUF DMA does the 128x128 SEG-blocked transpose; two
     local_scatters expand into the node-run-aligned "slotted" grid.
  3. The local-side score is added as a per-k-tile-segment broadcast;
     Lrelu+Exp on ACT gives w; duplicate (node,rel) instances are merged
     with shift-adds masked by a shipped dup-indicator grid.
  4. local_scatter builds B^T k-tiles [128 nodes, rel] (bf16); PE contracts
     feats^T against B accumulating U [feat, rel] in PSUM; a 1-wide ones
     matmul gives z[rel].
  5. One ReduceScatter over [U0|U1|z0|z1] gives each core its 125-relation
     shard; it finalizes out_shard = U1/z1 + U2/z2, transposes to
     [125, 128] and the host concatenates the 8 shards.

Host-side work is index bookkeeping only (sharding/sorting/layout); all
floating-point math happens on device.
"""

import os

import numpy as np

N = 50000
E = 1600000
EH = 256
RH = 128
R = 1000
EPS = 1e-16
NC = 8
SH = N // NC          # 6250 owned nodes per core
KT = 49               # k-tiles per core
NSH = KT * 128        # 6272 padded nodes per core
NTAB = NC * NSH       # padded global score-table rows
LS_MAX = 2000         # local_scatter dst free-dim limit we use
WIN = NTAB // 128     # score-table window per partition (392 nodes)
RSH = R // NC         # 125 relations finalized per core


def _ceil_even(x):
    return int(x + (x & 1))


def _rank_within_groups(key_sorted):
    """key_sorted: int array, sorted so equal keys are consecutive.
    Returns rank of each element within its group of equal keys."""
    n = key_sorted.shape[0]
    if n == 0:
        return np.zeros(0, dtype=np.int64)
    new = np.empty(n, dtype=bool)
    new[0] = True
    new[1:] = key_sorted[1:] != key_sorted[:-1]
    idx = np.arange(n)
    start = np.maximum.accumulate(np.where(new, idx, 0))
    return idx - start


def _host_prep(edge_index, rel):
    """Pure index bookkeeping. Returns per-core input arrays + static dims."""
    h = np.asarray(edge_index[0], dtype=np.int64)
    t = np.asarray(edge_index[1], dtype=np.int64)
    r_all = np.asarray(rel, dtype=np.int64)

    # term tau: own node (shard key), remote node, local col, remote col
    terms = [
        (h, t, 0, 1),  # e1 = s_h1[h] + s_h2[t]
        (t, h, 3, 2),  # e2 = s_t1[h] + s_t2[t] ; own=t local s_t2, remote s_t1
    ]

    percore = [[None, None] for _ in range(NC)]  # [core][term] -> dict
    for ti, (own, rem, lcol, rcol) in enumerate(terms):
        c_of = own // SH
        for c in range(NC):
            sel = c_of == c
            nl = (own[sel] - SH * c).astype(np.int64)
            rr = r_all[sel]
            rm = rem[sel]
            k = nl >> 7
            p = nl & 127
            # canonical order: (p, k, r) so partitions are grouped for compact
            order = np.lexsort((rr, k, p))
            k, p, rr, rm = k[order], p[order], rr[order], rm[order]
            percore[c][ti] = dict(k=k, p=p, r=rr, rm=rm)

    # common static dims
    S = np.zeros((2, KT), dtype=np.int64)  # seg widths per term
    Wc = 0
    Gmax = 1
    for c in range(NC):
        for ti in range(2):
            d = percore[c][ti]
            gid = d["p"] * KT + d["k"]
            cnt_kp = np.bincount(d["k"] * 128 + d["p"], minlength=KT * 128)
            S[ti] = np.maximum(S[ti], cnt_kp.reshape(KT, 128).max(axis=1))
            cnt_p = np.bincount(d["p"], minlength=128)
            Wc = max(Wc, int(cnt_p.max()))
            # dup groups: same (p,k,r)
            key = (gid * 1024 + d["r"]).astype(np.int64)
            occ = _rank_within_groups(key)
            d["occ"] = occ
            d["gid"] = gid
            Gmax = max(Gmax, int(occ.max()) + 1 if occ.size else 1)
    S = np.array([[_ceil_even(max(int(S[ti].max()), 2))] * KT
                  for ti in range(2)])
    Wc = _ceil_even(Wc + 8)
    seg_off = np.zeros((2, KT + 1), dtype=np.int64)
    seg_off[:, 1:] = np.cumsum(S, axis=1)
    W = [int(seg_off[ti, -1]) for ti in range(2)]
    # chunk split for compact->slotted local_scatter (dst free <= LS_MAX+47)
    kc = [0, 0]
    for ti in range(2):
        ks = int(np.searchsorted(seg_off[ti], W[ti] // 2))
        assert seg_off[ti, ks] <= 2040 and W[ti] - seg_off[ti, ks] <= 2040, (
            ti, W[ti], seg_off[ti, ks])
        kc[ti] = ks

    ins = []
    dims2 = {}
    for c in range(NC):
        m = {}
        for ti in range(2):
            d = percore[c][ti]
            k, p, rr, rm, occ = d["k"], d["p"], d["r"], d["rm"], d["occ"]
            # slot position within (k,p) segment (rank over (p,k) groups, r-sorted)
            j = _rank_within_groups(p * KT + k)
            slot = seg_off[ti, k] + j
            # ---- routed remote-score delivery ----
            # table layout matches the AllGather of scols_b [128, KT*4]
            # directly (contiguous sag DMA): entry for remote node
            # (core, k, p) at table row (core*128+p)//8, col
            # ((core*128+p)%8)*196 + k*4 + rcol
            rcol = terms[ti][3]
            crm = rm // SH
            nlr = rm - SH * crm
            cp = crm * 128 + (nlr & 127)
            ng = cp * KT + (nlr >> 7)             # remote node, table order
            psrc = (cp >> 3).astype(np.int64)     # table row
            # order instances by (psrc, ng) for run structure
            o2 = np.lexsort((ng, psrc))
            inv = np.empty_like(o2); inv[o2] = np.arange(len(o2))
            rpos = _rank_within_groups(psrc[o2])[inv]       # pos within psrc row
            posrun = _rank_within_groups((psrc * NTAB + ng)[o2])[inv]
            W2 = dims2.setdefault("W2", 0)
            dims2["W2"] = max(W2, int(rpos.max(initial=0)) + 1)
            # run starts: table entry -> first slot in remote-sorted grid
            runstart = np.full((128, 4 * WIN), -1, dtype=np.int16)
            hd = posrun == 0
            ei = (cp[hd] & 7) * (KT * 4) + (nlr[hd] >> 7) * 4 + rcol
            runstart[psrc[hd], ei] = rpos[hd].astype(np.int16)
            # stage-A: group by dest partition within each psrc row
            oA = np.lexsort((p, psrc))
            invA = np.empty_like(oA); invA[oA] = np.arange(len(oA))
            j2 = _rank_within_groups((psrc * 128 + p)[oA])[invA]
            dims2["SEG"] = max(dims2.setdefault("SEG", 0),
                               int(j2.max(initial=0)) + 1)
            d["psrc"] = psrc; d["rpos"] = rpos; d["posrun"] = posrun
            d["j2"] = j2; d["slot"] = slot
            # dup-indicator grid (bf16 1.0 at non-head slots) + scatter ids
            ngrid = np.zeros((128, W[ti]), dtype=np.float32)
            head = occ == 0
            nh = ~head
            if nh.any():
                ngrid[p[nh], slot[nh]] = 1.0
            scat = np.full((128, W[ti]), -1, dtype=np.int16)
            colid = (rr + 1000 * (k & 1)).astype(np.int16)
            scat[p[head], slot[head]] = colid[head]
            m[f"runstart{ti}"] = runstart
            m[f"ngrid{ti}"] = ngrid.astype("bfloat16")
            m[f"scat{ti}"] = scat
        ins.append(m)

    W2 = _ceil_even(dims2["W2"])
    SEG = _ceil_even(dims2["SEG"])
    NPASS = max(int(np.ceil(np.log2(max(
        max(int(percore[c][ti]["posrun"].max(initial=0)) + 1
            for c in range(NC) for ti in range(2)), 2)))), 1)
    G2W = 128 * SEG
    # split points for stage-A dst (G2W) and stage-C dst (W)
    for c in range(NC):
        m = ins[c]
        for ti in range(2):
            d = percore[c][ti]
            psrc, rpos, posrun, j2, slot = (d[x] for x in
                                            ("psrc", "rpos", "posrun",
                                             "j2", "slot"))
            p = d["p"]
            fills = np.zeros((NPASS, 128, W2), dtype=np.float32)
            for i in range(NPASS):
                s2 = posrun >= (1 << i)
                fills[i, psrc[s2], rpos[s2]] = 1.0
            m[f"fills{ti}"] = fills.astype("bfloat16")
            # stage A: tgrid[psrc, rpos] -> g2[psrc, j2*128 + pdst], 2 chunks
            # (s-major so the 128x128 transpose blocks are contiguous)
            half = (64 * SEG)
            dstA = (j2 * 128 + p)
            sa1 = np.full((128, W2), -1, dtype=np.int16)
            sa2 = np.full((128, W2), -1, dtype=np.int16)
            s1 = dstA < half
            sa1[psrc[s1], rpos[s1]] = dstA[s1].astype(np.int16)
            sa2[psrc[~s1], rpos[~s1]] = (dstA[~s1] - half).astype(np.int16)
            m[f"sa1_{ti}"] = sa1
            m[f"sa2_{ti}"] = sa2
            # stage C: g3[p, j2*128 + psrc] -> slotted[p, slot], 2 chunks
            so = int(seg_off[ti, kc[ti]])
            srcC = j2 * 128 + psrc
            sc1 = np.full((128, G2W), -1, dtype=np.int16)
            sc2 = np.full((128, G2W), -1, dtype=np.int16)
            cA = slot < so
            sc1[p[cA], srcC[cA]] = slot[cA].astype(np.int16)
            sc2[p[~cA], srcC[~cA]] = (slot[~cA] - so).astype(np.int16)
            m[f"sc1_{ti}"] = sc1
            m[f"sc2_{ti}"] = sc2
    dims = dict(S=S, seg_off=seg_off, W=W, Wc=Wc, Gmax=Gmax, kc=kc,
                W2=W2, SEG=SEG, NPASS=NPASS)
    return ins, dims


def _host_weights(W_h, W_t, a_h1, a_h2, a_t1, a_t2):
    Wcat = np.zeros((EH, 262), dtype=np.float32)
    Wcat[:, 1:129] = W_h
    Wcat[:, 130:258] = W_t
    Wcat[:, 258] = W_h @ a_h1
    Wcat[:, 259] = W_t @ a_h2
    Wcat[:, 260] = W_h @ a_t1
    Wcat[:, 261] = W_t @ a_t2
    return Wcat


def _numpy_sim(x_e, edge_index, rel, W_h, W_t, a_h1, a_h2, a_t1, a_t2):
    """Simulate the exact device algorithm (incl. bf16 rounding points) in
    numpy, to validate the host index bookkeeping without compiling."""
    import ml_dtypes  # noqa

    bf = lambda a: a.astype("bfloat16")
    f32 = lambda a: np.asarray(a, dtype=np.float32)
    ins, dims = _host_prep(edge_index, rel)
    Wcat = _host_weights(W_h, W_t, a_h1, a_h2, a_t1, a_t2)
    S, seg_off, W, kc = (dims[x] for x in ("S", "seg_off", "W", "kc"))
    # per-core projection + s-tables
    stab = np.zeros((NTAB, 4), dtype=np.float32)
    stab2 = np.zeros((NC * 128, KT * 4), dtype=np.float32)  # table layout
    xg = []
    for c in range(NC):
        xs = np.zeros((NSH, EH), dtype=np.float32)
        xs[:SH] = x_e[c * SH:(c + 1) * SH]
        proj = f32(bf(xs) @ bf(Wcat))  # psum f32 from bf16 operands
        stab[c * NSH:c * NSH + NSH] = proj[:, 258:262]
        stab2[c * 128:(c + 1) * 128] = (
            f32(bf(proj[:, 258:262])).reshape(KT, 128, 4)
            .transpose(1, 0, 2).reshape(128, KT * 4))
        g = bf(proj)
        g[:, 0] = 1.0
        g[:, 129] = 1.0
        xg.append(g)

    W2, SEG, NPASS = dims["W2"], dims["SEG"], dims["NPASS"]
    G2W = 128 * SEG

    def lscat(data, idxs, nelem):
        dst = np.zeros((128, nelem), dtype=data.dtype)
        pp, cc2 = np.where(idxs >= 0)
        dst[pp, idxs[pp, cc2]] = data[pp, cc2]
        return dst

    acc = np.zeros((2, 8, 128, 129), dtype=np.float32)
    stab_b = bf(stab2)
    for c in range(NC):
        for ti in range(2):
            tslice4 = stab_b.reshape(128, 4 * WIN)
            tgrid = lscat(tslice4, ins[c][f"runstart{ti}"], W2)
            fills = ins[c][f"fills{ti}"]
            for i in range(NPASS):
                sh = 1 << i
                upd = np.zeros_like(tgrid)
                upd[:, sh:] = bf(f32(fills[i][:, sh:]) * f32(tgrid[:, :-sh]))
                tgrid = bf(f32(tgrid) + f32(upd))
            g2a = lscat(tgrid, ins[c][f"sa1_{ti}"], 64 * SEG)
            g2b = lscat(tgrid, ins[c][f"sa2_{ti}"], G2W - 64 * SEG)
            g2 = np.concatenate([g2a, g2b], axis=1)
            g3 = g2.reshape(128, SEG, 128).transpose(2, 1, 0).reshape(128, G2W)
            so = seg_off[ti][kc[ti]]
            sla = lscat(g3, ins[c][f"sc1_{ti}"], so)
            slb = lscat(g3, ins[c][f"sc2_{ti}"], W[ti] - so)
            slotted = np.concatenate([sla, slb], axis=1)
            # local per-seg scalar add (bf16 grid + bf16 scores)
            lcol = (0, 3)[ti]
            sc_b = bf(stab[c * NSH:(c + 1) * NSH].reshape(KT, 128, 4)
                      .transpose(1, 0, 2)[:, :, lcol])      # [128, KT]
            Su = int(S[ti][0])
            slotted = bf(
                f32(slotted.reshape(128, KT, Su)) +
                f32(sc_b[:, :, None])).reshape(128, W[ti])
            e = np.where(slotted >= 0, f32(slotted), 0.01 * f32(slotted))
            wb = bf(np.exp(f32(e)))
            # merge dups: t += Ng[j+1]*t[j+1]; t += Ng[j+1]*Ng[j+2]*t[j+2]
            Ng = f32(ins[c][f"ngrid{ti}"])
            Wt = W[ti]
            t = f32(wb).copy()
            t[:, :Wt - 1] = f32(bf(t[:, :Wt - 1]) +
                                bf(f32(bf(Ng[:, 1:] * f32(wb[:, 1:])))))
            mb = Ng[:, 1:Wt - 1] * Ng[:, 2:]
            t[:, :Wt - 2] = f32(bf(t[:, :Wt - 2]) +
                                bf(f32(bf(mb * t[:, 2:]))))
            accw = bf(t)
            scat = ins[c][f"scat{ti}"]
            rhs_base = 0 if ti == 0 else 129
            for jj in range((KT + 1) // 2):
                k0 = 2 * jj
                sl = slice(seg_off[ti][k0], seg_off[ti][min(k0 + 2, KT)])
                B = np.zeros((128, LS_MAX), dtype="bfloat16")
                pp, cc2 = np.where(scat[:, sl] >= 0)
                B[pp, scat[:, sl][pp, cc2]] = accw[:, sl][pp, cc2]
                for k in (k0, k0 + 1):
                    if k >= KT:
                        continue
                    Bk = B[:, 1000 * (k & 1):1000 * (k & 1) + 1000]
                    rhs = xg[c][k * 128:(k + 1) * 128,
                                rhs_base:rhs_base + 129]
                    prod = f32(Bk).T @ f32(rhs)  # [1000, 129]
                    for mm in range(8):
                        M = 128 if mm < 7 else 104
                        acc[ti, mm, :M] += prod[mm * 128:mm * 128 + M]
    out = np.zeros((R, RH), dtype=np.float32)
    for mm in range(8):
        M = 128 if mm < 7 else 104
        for ti in range(2):
            z = acc[ti, mm, :M, 0]
            out[mm * 128:mm * 128 + M] += acc[ti, mm, :M, 1:] / (
                z[:, None] + EPS)
    return out


# ---------------------------------------------------------------------------
# Bass kernel
# ---------------------------------------------------------------------------

def _build_bass(dims):
    import concourse.bacc as bacc
    import concourse.tile as tile
    import concourse.mybir as mybir
    from concourse.tile_rust import add_dep_helper

    S, seg_off, W, kc = (dims[x] for x in ("S", "seg_off", "W", "kc"))
    assert dims["Gmax"] <= 4, dims["Gmax"]  # 2-pass dup merge limit
    S = np.asarray(S)
    f32 = mybir.dt.float32
    bf16 = mybir.dt.bfloat16
    i16 = mybir.dt.int16

    nc = bacc.Bacc("TRN2", target_bir_lowering=False, debug=False,
                   num_devices=NC)

    # --- dram tensors ---
    xT = nc.dram_tensor("xT", [EH, NSH], bf16, kind="ExternalInput")
    Wct = nc.dram_tensor("Wct", [EH, 262], bf16, kind="ExternalInput")
    W2, SEG, NPASS = dims["W2"], dims["SEG"], dims["NPASS"]
    G2W = 128 * SEG
    TS4 = 4 * (NTAB // 128)
    runstart = [nc.dram_tensor(f"runstart{t}", [128, TS4], i16,
                               kind="ExternalInput") for t in range(2)]
    fills = [nc.dram_tensor(f"fills{t}", [NPASS, 128, W2], bf16,
                            kind="ExternalInput") for t in range(2)]
    sa = [[nc.dram_tensor(f"sa{j}_{t}", [128, W2], i16,
                          kind="ExternalInput") for j in (1, 2)]
          for t in range(2)]
    sc = [[nc.dram_tensor(f"sc{j}_{t}", [128, G2W], i16,
                          kind="ExternalInput") for j in (1, 2)]
          for t in range(2)]
    ngrid = [nc.dram_tensor(f"ngrid{t}", [128, W[t]], bf16,
                            kind="ExternalInput") for t in range(2)]
    scat = [nc.dram_tensor(f"scat{t}", [128, W[t]], i16, kind="ExternalInput")
            for t in range(2)]
    out_d = nc.dram_tensor("out", [RSH, RH], f32, kind="ExternalOutput")

    sag_in = nc.dram_tensor("sag_in", [128, KT * 4], bf16)
    stab = nc.dram_tensor("stab", [NTAB * 4, 1], bf16,
                          addr_space="Shared")
    # per-term ReduceScatter payload: 8 shards of [U | z] per 125 rels
    SHW = 128 * RSH + RSH
    cc_in = [nc.dram_tensor(f"cc_in{t}", [NC * SHW, 1], f32)
             for t in range(2)]
    cc_out = [nc.dram_tensor(f"cc_out{t}", [SHW, 1], f32) for t in range(2)]
    groups = [list(range(NC))]

    with tile.TileContext(nc) as tc:
        with tc.tile_pool(name="persist", bufs=1) as pp_, \
             tc.tile_pool(name="io", bufs=1) as iop, \
             tc.tile_pool(name="io2", bufs=2) as io2, \
             tc.tile_pool(name="work", bufs=1) as wp, \
             tc.tile_pool(name="bt", bufs=2) as btp, \
             tc.tile_pool(name="acc_ps", bufs=1, space="PSUM") as aps:

            # ---- phase 1: projection ----
            xg = pp_.tile([128, KT * 262], bf16, tag="xg")
            scols_b = pp_.tile([128, KT * 4], bf16, tag="scolsb")
            wct_t = pp_.tile([128, 2 * 262], bf16, tag="wct")
            nc.sync.dma_start(
                out=wct_t[:].rearrange("p (a c) -> p a c", a=2),
                in_=Wct.ap().rearrange("(a p) c -> p a c", p=128))
            # batched xT loads (8 m-blocks per DMA = 2KB descriptors),
            # resident for both passes
            xts = []
            NG = (KT + 7) // 8
            for g in range(NG):
                nb = min(8, KT - g * 8) * 128
                xt_t = iop.tile([128, 2 * 1024], bf16, tag=f"xt8_{g}",
                                name=f"xt8_{g}")
                nc.sync.dma_start(
                    out=xt_t[:, 0:2 * nb].rearrange(
                        "p (a n) -> p a n", a=2),
                    in_=xT.ap()[:, g * 1024:g * 1024 + nb].rearrange(
                        "(a p) n -> p a n", p=128))
                xts.append(xt_t)

            def xt_slice(m, k2):
                t = xts[m // 8]
                o = (m % 8) * 128
                nb = min(8, KT - (m // 8) * 8) * 128
                return t[:, k2 * nb + o:k2 * nb + o + 128]

            # score columns first, so the AllGather launches early
            for m in range(KT):
                ps = aps.tile([128, 1024], f32,
                              tag=("psU", "psz")[m % 2], name="pss")
                for k2 in range(2):
                    nc.tensor.matmul(
                        ps[:, 0:4], xt_slice(m, k2),
                        wct_t[:, k2 * 262 + 258:k2 * 262 + 262],
                        start=(k2 == 0), stop=(k2 == 1))
                nc.vector.tensor_copy(
                    out=scols_b[:, m * 4:(m + 1) * 4], in_=ps[:, 0:4])
            sag_dma = nc.sync.dma_start(out=sag_in.ap(), in_=scols_b[:])
            nc.gpsimd.collective_compute(
                "AllGather", mybir.AluOpType.bypass, replica_groups=groups,
                ins=[sag_in.ap()], outs=[stab.ap()])
            # feature projections (overlap the AllGather), same xT tiles
            for m in range(KT):
                ps = aps.tile([128, 1024], f32,
                              tag=("psU", "psz")[m % 2], name="ps")
                for k2 in range(2):
                    nc.tensor.matmul(
                        ps[:, 0:258], xt_slice(m, k2),
                        wct_t[:, k2 * 262:k2 * 262 + 258],
                        start=(k2 == 0), stop=(k2 == 1))
                nc.scalar.copy(
                    out=xg[:, m * 262:m * 262 + 258], in_=ps[:, 0:258])
            xg3 = xg[:].rearrange("p (k c) -> p k c", c=262)
            nc.vector.memset(xg3[:, :, 0:1], 1.0)
            nc.vector.memset(xg3[:, :, 129:130], 1.0)

            # ---- phase 2+3: per-term edge pipeline ----
            # GPSIMD executes in issue order, so term1's route stages are
            # ISSUED between term0's B-phase scatter chunks to keep the
            # engine saturated.
            res = pp_.tile([128, 2000], f32, tag="res")
            onescol = pp_.tile([128, 2], bf16, tag="onescol")
            nc.vector.memset(onescol[:], 1.0)
            zrow = [None, None]
            from concourse.masks import make_identity
            identb = pp_.tile([128, 128], bf16, tag="identb")
            make_identity(nc, identb[:])

            def term_tables(ti, defer):
                # each dma_start costs ~600ns of serial dispatch on the
                # Sync sequencer, and an unsatisfied wait head-of-line
                # blocks all later triggers -- so issue order here IS the
                # execution order, and reused-buffer loads are deferred
                Wt = W[ti]
                d = {}
                d["rs"] = io2.tile([128, TS4], i16, tag="rs", name=f"rs{ti}")
                nc.sync.dma_start(out=d["rs"][:], in_=runstart[ti].ap())
                d["sa"] = [io2.tile([128, W2], i16, tag=f"sa{j}",
                                    name=f"sa{j}_{ti}") for j in range(2)]
                nc.sync.dma_start(out=d["sa"][0][:], in_=sa[ti][0].ap())
                nc.sync.dma_start(out=d["sa"][1][:], in_=sa[ti][1].ap())
                d["scat"] = io2.tile([128, Wt], i16, tag="scat",
                                     name=f"scat{ti}")
                nc.sync.dma_start(out=d["scat"][:], in_=scat[ti].ap())
                if not defer:
                    d["ng"] = io2.tile([128, Wt], bf16, tag="ngrid",
                                       name=f"ng{ti}", bufs=1)
                    nc.sync.dma_start(out=d["ng"][:], in_=ngrid[ti].ap())
                    term_tables_deferred(ti, d)
                return d

            def term_tables_deferred(ti, d):
                if "ng" not in d:
                    d["ng"] = io2.tile([128, W[ti]], bf16, tag="ngrid",
                                       name=f"ng{ti}", bufs=1)
                    nc.sync.dma_start(out=d["ng"][:], in_=ngrid[ti].ap())
                d["fl"] = io2.tile([128, NPASS * W2], bf16, tag="fl",
                                   name=f"fl{ti}", bufs=1)
                nc.sync.dma_start(
                    out=d["fl"][:].rearrange("p (g w) -> p g w", g=NPASS),
                    in_=fills[ti].ap().rearrange("g p w -> p g w"))
                d["sc"] = [io2.tile([128, G2W], i16, tag=f"sc{j}",
                                    name=f"sc{j}_{ti}", bufs=1)
                           for j in range(2)]
                nc.sync.dma_start(out=d["sc"][0][:], in_=sc[ti][0].ap())
                nc.sync.dma_start(out=d["sc"][1][:], in_=sc[ti][1].ap())

            def term_route_head(ti, d):
                # run-start scatter + log-doubling fill in table order
                tg = [io2.tile([128, W2], bf16, tag=f"tg{j}",
                               name=f"tg{j}_{ti}", bufs=2)
                      for j in range(2)]
                nc.gpsimd.local_scatter(
                    tg[0][:], tslice4[:], d["rs"][:],
                    channels=128, num_elems=W2, num_idxs=TS4)
                cur = 0
                for i in range(NPASS):
                    sh = 1 << i
                    x, y = tg[cur], tg[1 - cur]
                    nc.vector.tensor_tensor(
                        out=y[:, sh:W2],
                        in0=d["fl"][:, i * W2 + sh:(i + 1) * W2],
                        in1=x[:, 0:W2 - sh], op=mybir.AluOpType.mult)
                    nc.vector.tensor_tensor(
                        out=y[:, sh:W2], in0=y[:, sh:W2], in1=x[:, sh:W2],
                        op=mybir.AluOpType.add)
                    nc.vector.tensor_copy(out=y[:, 0:sh], in_=x[:, 0:sh])
                    cur = 1 - cur
                d["tgf"] = tg[cur]

            def term_stageA_T(ti, d):
                # stage A: group by destination partition
                g2 = io2.tile([128, G2W], bf16, tag="g2", name=f"g2_{ti}",
                              bufs=1)
                half = 64 * SEG
                nc.gpsimd.local_scatter(
                    g2[:, 0:half], d["tgf"][:], d["sa"][0][:],
                    channels=128, num_elems=half, num_idxs=W2)
                nc.gpsimd.local_scatter(
                    g2[:, half:G2W], d["tgf"][:], d["sa"][1][:],
                    channels=128, num_elems=G2W - half, num_idxs=W2)
                # SEG-blocked 128x128 transposes on the PE (no DMA triggers)
                g3 = io2.tile([128, G2W], bf16, tag="g3", name=f"g3_{ti}",
                              bufs=1)
                s = 0
                while s < SEG:
                    nblk = min(8, SEG - s)
                    ptr = aps.tile([128, 1024], bf16, tag="ptr", name="ptr",
                                   bufs=2)
                    for i in range(nblk):
                        nc.tensor.transpose(
                            out=ptr[:, i * 128:(i + 1) * 128],
                            in_=g2[:, (s + i) * 128:(s + i + 1) * 128],
                            identity=identb[:])
                    nc.scalar.copy(
                        out=g3[:, s * 128:(s + nblk) * 128],
                        in_=ptr[:, 0:nblk * 128])
                    s += nblk
                d["g3"] = g3

            def term_stageC_dve(ti, d):
                Wt = W[ti]
                # stage C: into node/rel slotted layout
                slotted = io2.tile([128, Wt], bf16, tag="slotted",
                                   name=f"slotted{ti}", bufs=1)
                so = int(seg_off[ti][kc[ti]])
                nc.gpsimd.local_scatter(
                    slotted[:, 0:so], d["g3"][:], d["sc"][0][:],
                    channels=128, num_elems=so, num_idxs=G2W)
                nc.gpsimd.local_scatter(
                    slotted[:, so:Wt], d["g3"][:], d["sc"][1][:],
                    channels=128, num_elems=Wt - so, num_idxs=G2W)
                # local-side add: broadcast each node's score over its
                # (uniform-width) k-tile segment with a step-0 AP
                lcol = (0, 3)[ti]
                Su = int(S[ti][0])
                sc_b = scols_b[:, lcol:4 * KT:4]
                nc.vector.tensor_tensor(
                    out=slotted[:].rearrange("p (k s) -> p k s", s=Su),
                    in0=slotted[:].rearrange("p (k s) -> p k s", s=Su),
                    in1=sc_b.to_broadcast([128, KT, Su]),
                    op=mybir.AluOpType.add)
                # w = exp(lrelu(e))
                nc.scalar.activation(slotted[:], slotted[:],
                                     mybir.ActivationFunctionType.Lrelu,
                                     alpha=0.01)
                nc.scalar.activation(slotted[:], slotted[:],
                                     mybir.ActivationFunctionType.Exp)
                wb = slotted
                # merge dups via shifted adds masked by the dup grid:
                #   accw[j] = w[j] + N[j+1]*w[j+1]
                #   accw[j] += N[j+1]N[j+2]*accw[j+2]
                accw = io2.tile([128, Wt], bf16, tag="accw", name=f"accw{ti}")
                tmp = io2.tile([128, Wt], bf16, tag="tmpm", name=f"tmpm{ti}",
                               bufs=1)
                ng_t = d["ng"]
                nc.vector.tensor_tensor(
                    out=tmp[:, 0:Wt - 1], in0=ng_t[:, 1:Wt], in1=wb[:, 1:Wt],
                    op=mybir.AluOpType.mult)
                nc.vector.tensor_tensor(
                    out=accw[:, 0:Wt - 1], in0=wb[:, 0:Wt - 1],
                    in1=tmp[:, 0:Wt - 1], op=mybir.AluOpType.add)
                nc.vector.tensor_copy(out=accw[:, Wt - 1:Wt],
                                      in_=wb[:, Wt - 1:Wt])
                nc.vector.tensor_tensor(
                    out=tmp[:, 0:Wt - 2], in0=ng_t[:, 1:Wt - 1],
                    in1=ng_t[:, 2:Wt], op=mybir.AluOpType.mult)
                nc.vector.tensor_tensor(
                    out=tmp[:, 0:Wt - 2], in0=tmp[:, 0:Wt - 2],
                    in1=accw[:, 2:Wt], op=mybir.AluOpType.mult)
                nc.vector.tensor_tensor(
                    out=accw[:, 0:Wt - 2], in0=accw[:, 0:Wt - 2],
                    in1=tmp[:, 0:Wt - 2], op=mybir.AluOpType.add)
                d["accw"] = accw

            def term_bphase(ti, d, inject=None):
                # B tiles + U^T / z matmuls (stream rel columns)
                accw, scat_t = d["accw"], d["scat"]
                feat_base = (1, 130)[ti]
                psU = aps.tile([128, 1024], f32, tag="psU",
                               name=f"psU{ti}")
                psz = aps.tile([1, 1024], f32, tag="psz",
                               name=f"psz{ti}")
                for jj in range((KT + 1) // 2):
                    k0 = 2 * jj
                    a = int(seg_off[ti][k0])
                    b = int(seg_off[ti][min(k0 + 2, KT)])
                    bt = btp.tile([128, LS_MAX], bf16, tag="bt")
                    nc.gpsimd.local_scatter(
                        bt[:], accw[:, a:b], scat_t[:, a:b],
                        channels=128, num_elems=LS_MAX, num_idxs=b - a)
                    for k in (k0, k0 + 1):
                        if k >= KT:
                            continue
                        off = 1000 * (k & 1)
                        feats = xg[:, k * 262 + feat_base:
                                   k * 262 + feat_base + 128]
                        for n0, n1 in ((0, 512), (512, 1000)):
                            nc.tensor.matmul(
                                psU[:, n0:n1], feats,
                                bt[:, off + n0:off + n1],
                                start=(k == 0), stop=(k == KT - 1))
                            nc.tensor.matmul(
                                psz[0:1, n0:n1], onescol[:, 0:1],
                                bt[:, off + n0:off + n1],
                                start=(k == 0), stop=(k == KT - 1))
                    if inject and jj in inject:
                        inject[jj]()
                nc.vector.tensor_copy(out=res[:, ti * 1000:ti * 1000 + 1000],
                                      in_=psU[:, 0:1000])
                zrow[ti] = pp_.tile([1, 1000], f32, tag=f"zrow{ti}",
                                    name=f"zrow{ti}")
                nc.vector.tensor_copy(out=zrow[ti][:], in_=psz[0:1, 0:1000])

            def term_rs(ti):
                # shard + reduce-scatter this term's [U | z] partials
                # (2 batched DMAs: [p, shard, w] strided into the 8 shards)
                nc.sync.dma_start(
                    out=cc_in[ti].ap().rearrange(
                        "(s x) 1 -> s x", s=NC)[:, 0:128 * RSH].rearrange(
                        "s (p w) -> p s w", p=128),
                    in_=res[:, ti * 1000:(ti + 1) * 1000].rearrange(
                        "p (s w) -> p s w", s=NC))
                nc.sync.dma_start(
                    out=cc_in[ti].ap().rearrange(
                        "(s x) 1 -> s x", s=NC)[:, 128 * RSH:SHW].rearrange(
                        "s (o w) -> o s w", o=1),
                    in_=zrow[ti][:].rearrange("o (s w) -> o s w", s=NC))
                nc.gpsimd.collective_compute(
                    "ReduceScatter", mybir.AluOpType.add,
                    replica_groups=groups,
                    ins=[cc_in[ti].ap()], outs=[cc_out[ti].ap()])

            red = pp_.tile([128, 2 * RSH], f32, tag="red")
            zt = pp_.tile([128, 2 * RSH], f32, tag="zt")

            def term_fin(ti):
                # load this term's reduced shard and divide by z
                nc.sync.dma_start(
                    out=red[:, ti * RSH:(ti + 1) * RSH],
                    in_=cc_out[ti].ap()[0:128 * RSH, :].rearrange(
                        "(p w) 1 -> p w", p=128))
                nc.sync.dma_start(
                    out=zt[:, ti * RSH:(ti + 1) * RSH],
                    in_=cc_out[ti].ap()[128 * RSH:, :].rearrange(
                        "(o w) 1 -> o w", o=1).to_broadcast([128, RSH]))
                sl = slice(ti * RSH, (ti + 1) * RSH)
                nc.vector.tensor_scalar_add(zt[:, sl], zt[:, sl], EPS)
                nc.vector.reciprocal(zt[:, sl], zt[:, sl])
                nc.vector.tensor_tensor(out=red[:, sl], in0=red[:, sl],
                                        in1=zt[:, sl],
                                        op=mybir.AluOpType.mult)

            d0 = term_tables(0, defer=False)
            d1 = term_tables(1, defer=True)
            # tslice4 last among head DMAs: its trigger waits on the
            # AllGather and would head-of-line block the table loads
            tslice4 = pp_.tile([128, TS4], bf16, tag="tslice4")
            nc.sync.dma_start(
                out=tslice4[:],
                in_=stab.ap().rearrange("(p w) 1 -> p w", p=128))
            term_route_head(0, d0)
            term_stageA_T(0, d0)
            term_stageC_dve(0, d0)
            term_bphase(0, d0, inject={
                0: lambda: term_tables_deferred(1, d1),
                1: lambda: term_route_head(1, d1),
                7: lambda: term_stageA_T(1, d1),
                13: lambda: term_stageC_dve(1, d1),
            })
            term_bphase(1, d1, inject={
                2: lambda: term_rs(0),
                16: lambda: term_fin(0),
            })
            term_rs(1)

            # ---- phase 4: finalize own 125-rel shard ----
            term_fin(1)
            nc.vector.tensor_tensor(out=red[:, 0:RSH], in0=red[:, 0:RSH],
                                    in1=red[:, RSH:2 * RSH],
                                    op=mybir.AluOpType.add)
            # transpose [128 feats, 125 rels] -> out [125, 128]
            ident = pp_.tile([128, 128], f32, tag="ident")
            make_identity(nc, ident[:])
            otile = pp_.tile([128, 128], f32, tag="otile")
            pst = aps.tile([128, 1024], f32, tag="psU", name="pst")
            nc.tensor.transpose(
                out=pst[:RSH, 0:128], in_=red[:, 0:RSH],
                identity=ident[:])
            nc.vector.tensor_copy(out=otile[:RSH, :], in_=pst[:RSH, 0:128])
            nc.sync.dma_start(out=out_d.ap()[:, :], in_=otile[:RSH, :])

    nc.compile()
    return nc


_CACHE = {}


def kernel(x_e, edge_index, rel, W_h, W_t, a_h1, a_h2, a_t1, a_t2):
    import ml_dtypes  # noqa: F401  (bfloat16 dtype registration)
    from concourse.bass_utils import run_bass_kernel_spmd

    x_e = np.asarray(x_e, dtype=np.float32)
    ins, dims = _host_prep(edge_index, rel)
    Wcat = _host_weights(
        np.asarray(W_h, np.float32), np.asarray(W_t, np.float32),
        np.asarray(a_h1, np.float32), np.asarray(a_h2, np.float32),
        np.asarray(a_t1, np.float32), np.asarray(a_t2, np.float32))

    key = (tuple(dims["W"]), dims["Wc"], dims["Gmax"], tuple(dims["kc"]),
           tuple(map(tuple, dims["S"])), dims["W2"], dims["SEG"],
           dims["NPASS"], "v2")
    if key not in _CACHE:
        _CACHE[key] = _build_bass(dims)
    nc = _CACHE[key]

    wct_b = Wcat.astype("bfloat16")
    in_maps = []
    for c in range(NC):
        xs = np.zeros((NSH, EH), dtype=np.float32)
        xs[:SH] = x_e[c * SH:(c + 1) * SH]
        m = {"xT": np.ascontiguousarray(xs.T).astype("bfloat16"),
             "Wct": wct_b}
        for ti in range(2):
            for nm in ("runstart", "ngrid", "scat", "fills"):
                m[f"{nm}{ti}"] = ins[c][f"{nm}{ti}"]
            for j in (1, 2):
                m[f"sa{j}_{ti}"] = ins[c][f"sa{j}_{ti}"]
                m[f"sc{j}_{ti}"] = ins[c][f"sc{j}_{ti}"]
        in_maps.append(m)

    trace = bool(int(os.environ.get("GAT_TRACE", "0")))
    res = run_bass_kernel_spmd(nc, in_maps, core_ids=list(range(NC)),
                               trace=trace)
    kernel.last_exec_time_ns = res.exec_time_ns
    kernel.last_results = res
    out = np.concatenate(
        [np.asarray(res.results[c]["out"], dtype=np.float32)
         for c in range(NC)], axis=0)
    return out


kernel.last_exec_time_ns = None


# revision 55
# speedup vs baseline: 1.1316x; 1.1316x over previous
"""GAT edge->relation aggregation on 8 trn2 NeuronCores.

Strategy (node-sharded, matmul-centric):
  out_tau[r] = (1/z_tau[r]) * sum_e w_tau_e * feat_tau[node_e],  tau in {1,2}
with w_tau_e = exp(leakyrelu(s_a[own_e] + s_b[rem_e])).  Softmax max-subtraction
cancels algebraically (z >> EPS), so it is skipped.

Per core (owns 6250 nodes, padded to 6272 = 49*128):
  1. One matmul pass over x_shard gives the four per-node scalar scores
     (weights folded host-side: W_h@a_h1 etc) -> AllGather of the [6272,4]
     score table; a second pass (same resident xT tiles) gives the feature
     projections [x_r_h | x_r_t].
  2. Edge instances are sharded by owning node (h for term1, t for term2).
     Remote-side scores are delivered via: run-start local_scatter from the
     gathered table into a compact per-partition grid; log-doubling fill
     (DVE) expands runs; two local_scatters group by dest partition; one
     SBUF->SBUF DMA does the 128x128 SEG-blocked transpose; two
     local_scatters expand into the node-run-aligned "slotted" grid.
  3. The local-side score is added as a per-k-tile-segment broadcast;
     Lrelu+Exp on ACT gives w; duplicate (node,rel) instances are merged
     with shift-adds masked by a shipped dup-indicator grid.
  4. local_scatter builds B^T k-tiles [128 nodes, rel] (bf16); PE contracts
     feats^T against B accumulating U [feat, rel] in PSUM; a 1-wide ones
     matmul gives z[rel].
  5. One ReduceScatter over [U0|U1|z0|z1] gives each core its 125-relation
     shard; it finalizes out_shard = U1/z1 + U2/z2, transposes to
     [125, 128] and the host concatenates the 8 shards.

Host-side work is index bookkeeping only (sharding/sorting/layout); all
floating-point math happens on device.
"""

import os

import numpy as np

N = 50000
E = 1600000
EH = 256
RH = 128
R = 1000
EPS = 1e-16
NC = 8
SH = N // NC          # 6250 owned nodes per core
KT = 49               # k-tiles per core
NSH = KT * 128        # 6272 padded nodes per core
NTAB = NC * NSH       # padded global score-table rows
LS_MAX = 2000         # local_scatter dst free-dim limit we use
WIN = NTAB // 128     # score-table window per partition (392 nodes)
RSH = R // NC         # 125 relations finalized per core


def _ceil_even(x):
    return int(x + (x & 1))


def _rank_within_groups(key_sorted):
    """key_sorted: int array, sorted so equal keys are consecutive.
    Returns rank of each element within its group of equal keys."""
    n = key_sorted.shape[0]
    if n == 0:
        return np.zeros(0, dtype=np.int64)
    new = np.empty(n, dtype=bool)
    new[0] = True
    new[1:] = key_sorted[1:] != key_sorted[:-1]
    idx = np.arange(n)
    start = np.maximum.accumulate(np.where(new, idx, 0))
    return idx - start


def _host_prep(edge_index, rel):
    """Pure index bookkeeping. Returns per-core input arrays + static dims."""
    h = np.asarray(edge_index[0], dtype=np.int64)
    t = np.asarray(edge_index[1], dtype=np.int64)
    r_all = np.asarray(rel, dtype=np.int64)

    # term tau: own node (shard key), remote node, local col, remote col
    terms = [
        (h, t, 0, 1),  # e1 = s_h1[h] + s_h2[t]
        (t, h, 3, 2),  # e2 = s_t1[h] + s_t2[t] ; own=t local s_t2, remote s_t1
    ]

    percore = [[None, None] for _ in range(NC)]  # [core][term] -> dict
    for ti, (own, rem, lcol, rcol) in enumerate(terms):
        c_of = own // SH
        for c in range(NC):
            sel = c_of == c
            nl = (own[sel] - SH * c).astype(np.int64)
            rr = r_all[sel]
            rm = rem[sel]
            k = nl >> 7
            p = nl & 127
            # canonical order: (p, k, r) so partitions are grouped for compact
            order = np.lexsort((rr, k, p))
            k, p, rr, rm = k[order], p[order], rr[order], rm[order]
            percore[c][ti] = dict(k=k, p=p, r=rr, rm=rm)

    # common static dims
    S = np.zeros((2, KT), dtype=np.int64)  # seg widths per term
    Wc = 0
    Gmax = 1
    for c in range(NC):
        for ti in range(2):
            d = percore[c][ti]
            gid = d["p"] * KT + d["k"]
            cnt_kp = np.bincount(d["k"] * 128 + d["p"], minlength=KT * 128)
            S[ti] = np.maximum(S[ti], cnt_kp.reshape(KT, 128).max(axis=1))
            cnt_p = np.bincount(d["p"], minlength=128)
            Wc = max(Wc, int(cnt_p.max()))
            # dup groups: same (p,k,r)
            key = (gid * 1024 + d["r"]).astype(np.int64)
            occ = _rank_within_groups(key)
            d["occ"] = occ
            d["gid"] = gid
            Gmax = max(Gmax, int(occ.max()) + 1 if occ.size else 1)
    S = np.array([[_ceil_even(max(int(S[ti].max()), 2))] * KT
                  for ti in range(2)])
    Wc = _ceil_even(Wc + 8)
    seg_off = np.zeros((2, KT + 1), dtype=np.int64)
    seg_off[:, 1:] = np.cumsum(S, axis=1)
    W = [int(seg_off[ti, -1]) for ti in range(2)]
    # chunk split for compact->slotted local_scatter (dst free <= LS_MAX+47)
    kc = [0, 0]
    for ti in range(2):
        ks = int(np.searchsorted(seg_off[ti], W[ti] // 2))
        assert seg_off[ti, ks] <= 2040 and W[ti] - seg_off[ti, ks] <= 2040, (
            ti, W[ti], seg_off[ti, ks])
        kc[ti] = ks

    ins = []
    dims2 = {}
    for c in range(NC):
        m = {}
        for ti in range(2):
            d = percore[c][ti]
            k, p, rr, rm, occ = d["k"], d["p"], d["r"], d["rm"], d["occ"]
            # slot position within (k,p) segment (rank over (p,k) groups, r-sorted)
            j = _rank_within_groups(p * KT + k)
            slot = seg_off[ti, k] + j
            # ---- routed remote-score delivery ----
            # table layout matches the AllGather of scols_b [128, KT*4]
            # directly (contiguous sag DMA): entry for remote node
            # (core, k, p) at table row (core*128+p)//8, col
            # ((core*128+p)%8)*196 + k*4 + rcol
            rcol = terms[ti][3]
            crm = rm // SH
            nlr = rm - SH * crm
            cp = crm * 128 + (nlr & 127)
            ng = cp * KT + (nlr >> 7)             # remote node, table order
            psrc = (cp >> 3).astype(np.int64)     # table row
            # order instances by (psrc, ng) for run structure
            o2 = np.lexsort((ng, psrc))
            inv = np.empty_like(o2); inv[o2] = np.arange(len(o2))
            rpos = _rank_within_groups(psrc[o2])[inv]       # pos within psrc row
            posrun = _rank_within_groups((psrc * NTAB + ng)[o2])[inv]
            W2 = dims2.setdefault("W2", 0)
            dims2["W2"] = max(W2, int(rpos.max(initial=0)) + 1)
            # run starts: table entry -> first slot in remote-sorted grid
            runstart = np.full((128, 4 * WIN), -1, dtype=np.int16)
            hd = posrun == 0
            ei = (cp[hd] & 7) * (KT * 4) + (nlr[hd] >> 7) * 4 + rcol
            runstart[psrc[hd], ei] = rpos[hd].astype(np.int16)
            # stage-A: group by dest partition within each psrc row
            oA = np.lexsort((p, psrc))
            invA = np.empty_like(oA); invA[oA] = np.arange(len(oA))
            j2 = _rank_within_groups((psrc * 128 + p)[oA])[invA]
            dims2["SEG"] = max(dims2.setdefault("SEG", 0),
                               int(j2.max(initial=0)) + 1)
            d["psrc"] = psrc; d["rpos"] = rpos; d["posrun"] = posrun
            d["j2"] = j2; d["slot"] = slot
            # dup-indicator grid (bf16 1.0 at non-head slots) + scatter ids
            ngrid = np.zeros((128, W[ti]), dtype=np.float32)
            head = occ == 0
            nh = ~head
            if nh.any():
                ngrid[p[nh], slot[nh]] = 1.0
            scat = np.full((128, W[ti]), -1, dtype=np.int16)
            colid = (rr + 1000 * (k & 1)).astype(np.int16)
            scat[p[head], slot[head]] = colid[head]
            m[f"runstart{ti}"] = runstart
            m[f"ngrid{ti}"] = ngrid.astype("bfloat16")
            m[f"scat{ti}"] = scat
        ins.append(m)

    W2 = _ceil_even(dims2["W2"])
    SEG = _ceil_even(dims2["SEG"])
    NPASS = max(int(np.ceil(np.log2(max(
        max(int(percore[c][ti]["posrun"].max(initial=0)) + 1
            for c in range(NC) for ti in range(2)), 2)))), 1)
    G2W = 128 * SEG
    # split points for stage-A dst (G2W) and stage-C dst (W)
    for c in range(NC):
        m = ins[c]
        for ti in range(2):
            d = percore[c][ti]
            psrc, rpos, posrun, j2, slot = (d[x] for x in
                                            ("psrc", "rpos", "posrun",
                                             "j2", "slot"))
            p = d["p"]
            fills = np.zeros((NPASS, 128, W2), dtype=np.float32)
            for i in range(NPASS):
                s2 = posrun >= (1 << i)
                fills[i, psrc[s2], rpos[s2]] = 1.0
            m[f"fills{ti}"] = fills.astype("bfloat16")
            # stage A: tgrid[psrc, rpos] -> g2[psrc, j2*128 + pdst], 2 chunks
            # (s-major so the 128x128 transpose blocks are contiguous)
            half = (64 * SEG)
            dstA = (j2 * 128 + p)
            sa1 = np.full((128, W2), -1, dtype=np.int16)
            sa2 = np.full((128, W2), -1, dtype=np.int16)
            s1 = dstA < half
            sa1[psrc[s1], rpos[s1]] = dstA[s1].astype(np.int16)
            sa2[psrc[~s1], rpos[~s1]] = (dstA[~s1] - half).astype(np.int16)
            m[f"sa1_{ti}"] = sa1
            m[f"sa2_{ti}"] = sa2
            # stage C: g3[p, j2*128 + psrc] -> slotted[p, slot], 2 chunks
            so = int(seg_off[ti, kc[ti]])
            srcC = j2 * 128 + psrc
            sc1 = np.full((128, G2W), -1, dtype=np.int16)
            sc2 = np.full((128, G2W), -1, dtype=np.int16)
            cA = slot < so
            sc1[p[cA], srcC[cA]] = slot[cA].astype(np.int16)
            sc2[p[~cA], srcC[~cA]] = (slot[~cA] - so).astype(np.int16)
            m[f"sc1_{ti}"] = sc1
            m[f"sc2_{ti}"] = sc2
    dims = dict(S=S, seg_off=seg_off, W=W, Wc=Wc, Gmax=Gmax, kc=kc,
                W2=W2, SEG=SEG, NPASS=NPASS)
    return ins, dims


def _host_weights(W_h, W_t, a_h1, a_h2, a_t1, a_t2):
    Wcat = np.zeros((EH, 262), dtype=np.float32)
    Wcat[:, 1:129] = W_h
    Wcat[:, 130:258] = W_t
    Wcat[:, 258] = W_h @ a_h1
    Wcat[:, 259] = W_t @ a_h2
    Wcat[:, 260] = W_h @ a_t1
    Wcat[:, 261] = W_t @ a_t2
    return Wcat


def _numpy_sim(x_e, edge_index, rel, W_h, W_t, a_h1, a_h2, a_t1, a_t2):
    """Simulate the exact device algorithm (incl. bf16 rounding points) in
    numpy, to validate the host index bookkeeping without compiling."""
    import ml_dtypes  # noqa

    bf = lambda a: a.astype("bfloat16")
    f32 = lambda a: np.asarray(a, dtype=np.float32)
    ins, dims = _host_prep(edge_index, rel)
    Wcat = _host_weights(W_h, W_t, a_h1, a_h2, a_t1, a_t2)
    S, seg_off, W, kc = (dims[x] for x in ("S", "seg_off", "W", "kc"))
    # per-core projection + s-tables
    stab = np.zeros((NTAB, 4), dtype=np.float32)
    stab2 = np.zeros((NC * 128, KT * 4), dtype=np.float32)  # table layout
    xg = []
    for c in range(NC):
        xs = np.zeros((NSH, EH), dtype=np.float32)
        xs[:SH] = x_e[c * SH:(c + 1) * SH]
        proj = f32(bf(xs) @ bf(Wcat))  # psum f32 from bf16 operands
        stab[c * NSH:c * NSH + NSH] = proj[:, 258:262]
        stab2[c * 128:(c + 1) * 128] = (
            f32(bf(proj[:, 258:262])).reshape(KT, 128, 4)
            .transpose(1, 0, 2).reshape(128, KT * 4))
        g = bf(proj)
        g[:, 0] = 1.0
        g[:, 129] = 1.0
        xg.append(g)

    W2, SEG, NPASS = dims["W2"], dims["SEG"], dims["NPASS"]
    G2W = 128 * SEG

    def lscat(data, idxs, nelem):
        dst = np.zeros((128, nelem), dtype=data.dtype)
        pp, cc2 = np.where(idxs >= 0)
        dst[pp, idxs[pp, cc2]] = data[pp, cc2]
        return dst

    acc = np.zeros((2, 8, 128, 129), dtype=np.float32)
    stab_b = bf(stab2)
    for c in range(NC):
        for ti in range(2):
            tslice4 = stab_b.reshape(128, 4 * WIN)
            tgrid = lscat(tslice4, ins[c][f"runstart{ti}"], W2)
            fills = ins[c][f"fills{ti}"]
            for i in range(NPASS):
                sh = 1 << i
                upd = np.zeros_like(tgrid)
                upd[:, sh:] = bf(f32(fills[i][:, sh:]) * f32(tgrid[:, :-sh]))
                tgrid = bf(f32(tgrid) + f32(upd))
            g2a = lscat(tgrid, ins[c][f"sa1_{ti}"], 64 * SEG)
            g2b = lscat(tgrid, ins[c][f"sa2_{ti}"], G2W - 64 * SEG)
            g2 = np.concatenate([g2a, g2b], axis=1)
            g3 = g2.reshape(128, SEG, 128).transpose(2, 1, 0).reshape(128, G2W)
            so = seg_off[ti][kc[ti]]
            sla = lscat(g3, ins[c][f"sc1_{ti}"], so)
            slb = lscat(g3, ins[c][f"sc2_{ti}"], W[ti] - so)
            slotted = np.concatenate([sla, slb], axis=1)
            # local per-seg scalar add (bf16 grid + bf16 scores)
            lcol = (0, 3)[ti]
            sc_b = bf(stab[c * NSH:(c + 1) * NSH].reshape(KT, 128, 4)
                      .transpose(1, 0, 2)[:, :, lcol])      # [128, KT]
            Su = int(S[ti][0])
            slotted = bf(
                f32(slotted.reshape(128, KT, Su)) +
                f32(sc_b[:, :, None])).reshape(128, W[ti])
            e = np.where(slotted >= 0, f32(slotted), 0.01 * f32(slotted))
            wb = bf(np.exp(f32(e)))
            # merge dups: t += Ng[j+1]*t[j+1]; t += Ng[j+1]*Ng[j+2]*t[j+2]
            Ng = f32(ins[c][f"ngrid{ti}"])
            Wt = W[ti]
            t = f32(wb).copy()
            t[:, :Wt - 1] = f32(bf(t[:, :Wt - 1]) +
                                bf(f32(bf(Ng[:, 1:] * f32(wb[:, 1:])))))
            mb = Ng[:, 1:Wt - 1] * Ng[:, 2:]
            t[:, :Wt - 2] = f32(bf(t[:, :Wt - 2]) +
                                bf(f32(bf(mb * t[:, 2:]))))
            accw = bf(t)
            scat = ins[c][f"scat{ti}"]
            rhs_base = 0 if ti == 0 else 129
            for jj in range((KT + 1) // 2):
                k0 = 2 * jj
                sl = slice(seg_off[ti][k0], seg_off[ti][min(k0 + 2, KT)])
                B = np.zeros((128, LS_MAX), dtype="bfloat16")
                pp, cc2 = np.where(scat[:, sl] >= 0)
                B[pp, scat[:, sl][pp, cc2]] = accw[:, sl][pp, cc2]
                for k in (k0, k0 + 1):
                    if k >= KT:
                        continue
                    Bk = B[:, 1000 * (k & 1):1000 * (k & 1) + 1000]
                    rhs = xg[c][k * 128:(k + 1) * 128,
                                rhs_base:rhs_base + 129]
                    prod = f32(Bk).T @ f32(rhs)  # [1000, 129]
                    for mm in range(8):
                        M = 128 if mm < 7 else 104
                        acc[ti, mm, :M] += prod[mm * 128:mm * 128 + M]
    out = np.zeros((R, RH), dtype=np.float32)
    for mm in range(8):
        M = 128 if mm < 7 else 104
        for ti in range(2):
            z = acc[ti, mm, :M, 0]
            out[mm * 128:mm * 128 + M] += acc[ti, mm, :M, 1:] / (
                z[:, None] + EPS)
    return out


# ---------------------------------------------------------------------------
# Bass kernel
# ---------------------------------------------------------------------------

def _build_bass(dims):
    import concourse.bacc as bacc
    import concourse.tile as tile
    import concourse.mybir as mybir
    from concourse.tile_rust import add_dep_helper

    S, seg_off, W, kc = (dims[x] for x in ("S", "seg_off", "W", "kc"))
    assert dims["Gmax"] <= 4, dims["Gmax"]  # 2-pass dup merge limit
    S = np.asarray(S)
    f32 = mybir.dt.float32
    bf16 = mybir.dt.bfloat16
    i16 = mybir.dt.int16

    nc = bacc.Bacc("TRN2", target_bir_lowering=False, debug=False,
                   num_devices=NC)

    # --- dram tensors ---
    xT = nc.dram_tensor("xT", [EH, NSH], bf16, kind="ExternalInput")
    Wct = nc.dram_tensor("Wct", [EH, 262], bf16, kind="ExternalInput")
    W2, SEG, NPASS = dims["W2"], dims["SEG"], dims["NPASS"]
    G2W = 128 * SEG
    TS4 = 4 * (NTAB // 128)
    runstart = [nc.dram_tensor(f"runstart{t}", [128, TS4], i16,
                               kind="ExternalInput") for t in range(2)]
    fills = [nc.dram_tensor(f"fills{t}", [NPASS, 128, W2], bf16,
                            kind="ExternalInput") for t in range(2)]
    sa = [[nc.dram_tensor(f"sa{j}_{t}", [128, W2], i16,
                          kind="ExternalInput") for j in (1, 2)]
          for t in range(2)]
    sc = [[nc.dram_tensor(f"sc{j}_{t}", [128, G2W], i16,
                          kind="ExternalInput") for j in (1, 2)]
          for t in range(2)]
    ngrid = [nc.dram_tensor(f"ngrid{t}", [128, W[t]], bf16,
                            kind="ExternalInput") for t in range(2)]
    scat = [nc.dram_tensor(f"scat{t}", [128, W[t]], i16, kind="ExternalInput")
            for t in range(2)]
    out_d = nc.dram_tensor("out", [RSH, RH], f32, kind="ExternalOutput")

    sag_in = nc.dram_tensor("sag_in", [128, KT * 4], bf16)
    stab = nc.dram_tensor("stab", [NTAB * 4, 1], bf16,
                          addr_space="Shared")
    # per-term ReduceScatter payload: 8 shards of [U | z] per 125 rels
    SHW = 128 * RSH + RSH
    cc_in = [nc.dram_tensor(f"cc_in{t}", [NC * SHW, 1], f32)
             for t in range(2)]
    cc_out = [nc.dram_tensor(f"cc_out{t}", [SHW, 1], f32) for t in range(2)]
    groups = [list(range(NC))]

    with tile.TileContext(nc) as tc:
        with tc.tile_pool(name="persist", bufs=1) as pp_, \
             tc.tile_pool(name="io", bufs=1) as iop, \
             tc.tile_pool(name="io2", bufs=2) as io2, \
             tc.tile_pool(name="work", bufs=1) as wp, \
             tc.tile_pool(name="bt", bufs=2) as btp, \
             tc.tile_pool(name="acc_ps", bufs=1, space="PSUM") as aps:

            # ---- phase 1: projection (xT tiles in a scoped pool so their
            # SBUF is released for the route double-buffers) ----
            xg = pp_.tile([128, KT * 262], bf16, tag="xg")
            scols_b = pp_.tile([128, KT * 4], bf16, tag="scolsb")
            wct_t = pp_.tile([128, 2 * 262], bf16, tag="wct")
            nc.sync.dma_start(
                out=wct_t[:].rearrange("p (a c) -> p a c", a=2),
                in_=Wct.ap().rearrange("(a p) c -> p a c", p=128))
            with tc.tile_pool(name="xtp", bufs=1) as xtp:
                # batched xT loads (8 m-blocks per DMA = 2KB descriptors),
                # resident for both passes
                xts = []
                NG = (KT + 7) // 8
                for g in range(NG):
                    nb = min(8, KT - g * 8) * 128
                    xt_t = xtp.tile([128, 2 * nb], bf16, tag=f"xt8_{g}",
                                    name=f"xt8_{g}")
                    nc.sync.dma_start(
                        out=xt_t[:, 0:2 * nb].rearrange(
                            "p (a n) -> p a n", a=2),
                        in_=xT.ap()[:, g * 1024:g * 1024 + nb].rearrange(
                            "(a p) n -> p a n", p=128))
                    xts.append(xt_t)

                def xt_slice(m, k2):
                    t = xts[m // 8]
                    o = (m % 8) * 128
                    nb = min(8, KT - (m // 8) * 8) * 128
                    return t[:, k2 * nb + o:k2 * nb + o + 128]

                # score columns first, so the AllGather launches early
                for m in range(KT):
                    ps = aps.tile([128, 1024], f32,
                                  tag=("psU", "psz")[m % 2], name="pss")
                    for k2 in range(2):
                        nc.tensor.matmul(
                            ps[:, 0:4], xt_slice(m, k2),
                            wct_t[:, k2 * 262 + 258:k2 * 262 + 262],
                            start=(k2 == 0), stop=(k2 == 1))
                    nc.vector.tensor_copy(
                        out=scols_b[:, m * 4:(m + 1) * 4], in_=ps[:, 0:4])
                nc.sync.dma_start(out=sag_in.ap(), in_=scols_b[:])
                nc.gpsimd.collective_compute(
                    "AllGather", mybir.AluOpType.bypass,
                    replica_groups=groups,
                    ins=[sag_in.ap()], outs=[stab.ap()])
                # feature projections (overlap the AllGather)
                for m in range(KT):
                    ps = aps.tile([128, 1024], f32,
                                  tag=("psU", "psz")[m % 2], name="ps")
                    for k2 in range(2):
                        nc.tensor.matmul(
                            ps[:, 0:258], xt_slice(m, k2),
                            wct_t[:, k2 * 262:k2 * 262 + 258],
                            start=(k2 == 0), stop=(k2 == 1))
                    nc.scalar.copy(
                        out=xg[:, m * 262:m * 262 + 258], in_=ps[:, 0:258])
            xg3 = xg[:].rearrange("p (k c) -> p k c", c=262)
            nc.vector.memset(xg3[:, :, 0:1], 1.0)
            nc.vector.memset(xg3[:, :, 129:130], 1.0)

            # ---- phase 2+3: per-term edge pipeline ----
            # GPSIMD executes in issue order, so term1's route stages are
            # ISSUED between term0's B-phase scatter chunks to keep the
            # engine saturated.
            res = pp_.tile([128, 2000], f32, tag="res")
            onescol = pp_.tile([128, 2], bf16, tag="onescol")
            nc.vector.memset(onescol[:], 1.0)
            zrowt = [pp_.tile([1, 1000], f32, tag=f"zrow{t}",
                              name=f"zrow{t}")
                     for t in range(2)]

            def zrow(ti):
                return zrowt[ti][:]
            from concourse.masks import make_identity
            identb = pp_.tile([128, 128], bf16, tag="identb")
            make_identity(nc, identb[:])

            def term_tables_pre(ti):
                # each dma_start costs ~600ns of serial dispatch on the
                # Sync sequencer, and an unsatisfied wait head-of-line
                # blocks all later triggers -- so issue order here IS the
                # execution order
                d = {}
                d["rs"] = io2.tile([128, TS4], i16, tag="rs", name=f"rs{ti}")
                nc.sync.dma_start(out=d["rs"][:], in_=runstart[ti].ap())
                d["sa"] = [io2.tile([128, W2], i16, tag=f"sa{j}",
                                    name=f"sa{j}_{ti}") for j in range(2)]
                nc.sync.dma_start(out=d["sa"][0][:], in_=sa[ti][0].ap())
                nc.sync.dma_start(out=d["sa"][1][:], in_=sa[ti][1].ap())
                return d

            def term_tables_post(ti, d):
                Wt = W[ti]
                d["sc"] = [io2.tile([128, G2W], i16, tag=f"sc{j}",
                                    name=f"sc{j}_{ti}", bufs=1)
                           for j in range(2)]
                nc.sync.dma_start(out=d["sc"][0][:], in_=sc[ti][0].ap())
                nc.sync.dma_start(out=d["sc"][1][:], in_=sc[ti][1].ap())
                d["ng"] = io2.tile([128, Wt], bf16, tag="ngrid",
                                   name=f"ng{ti}", bufs=1)
                nc.sync.dma_start(out=d["ng"][:], in_=ngrid[ti].ap())
                d["scat"] = io2.tile([128, Wt], i16, tag="scat",
                                     name=f"scat{ti}")
                nc.sync.dma_start(out=d["scat"][:], in_=scat[ti].ap())

            def term_route_head(ti, d):
                # fills table (single shared buffer; term1's load waits for
                # term0's last fill pass, covered by the stage-A window)
                d["fl"] = io2.tile([128, NPASS * W2], bf16, tag="fl",
                                   name=f"fl{ti}", bufs=1)
                nc.sync.dma_start(
                    out=d["fl"][:].rearrange("p (g w) -> p g w", g=NPASS),
                    in_=fills[ti].ap().rearrange("g p w -> p g w"))
                # run-start scatter + log-doubling fill in table order
                tg = [io2.tile([128, W2], bf16, tag=f"tg{j}",
                               name=f"tg{j}_{ti}", bufs=2)
                      for j in range(2)]
                nc.gpsimd.local_scatter(
                    tg[0][:], tslice4[:], d["rs"][:],
                    channels=128, num_elems=W2, num_idxs=TS4)
                cur = 0
                for i in range(NPASS):
                    sh = 1 << i
                    x, y = tg[cur], tg[1 - cur]
                    nc.vector.tensor_tensor(
                        out=y[:, sh:W2],
                        in0=d["fl"][:, i * W2 + sh:(i + 1) * W2],
                        in1=x[:, 0:W2 - sh], op=mybir.AluOpType.mult)
                    nc.vector.tensor_tensor(
                        out=y[:, sh:W2], in0=y[:, sh:W2], in1=x[:, sh:W2],
                        op=mybir.AluOpType.add)
                    nc.vector.tensor_copy(out=y[:, 0:sh], in_=x[:, 0:sh])
                    cur = 1 - cur
                d["tgf"] = tg[cur]

            def term_stageA_T(ti, d):
                # stage A: group by destination partition
                g2 = io2.tile([128, G2W], bf16, tag="g2", name=f"g2_{ti}",
                              bufs=2)
                half = 64 * SEG
                nc.gpsimd.local_scatter(
                    g2[:, 0:half], d["tgf"][:], d["sa"][0][:],
                    channels=128, num_elems=half, num_idxs=W2)
                nc.gpsimd.local_scatter(
                    g2[:, half:G2W], d["tgf"][:], d["sa"][1][:],
                    channels=128, num_elems=G2W - half, num_idxs=W2)
                # SEG-blocked 128x128 transposes on the PE (no DMA triggers)
                g3 = io2.tile([128, G2W], bf16, tag="g3", name=f"g3_{ti}",
                              bufs=2)
                s = 0
                while s < SEG:
                    nblk = min(8, SEG - s)
                    ptr = aps.tile([128, 1024], bf16, tag="ptr", name="ptr",
                                   bufs=2)
                    for i in range(nblk):
                        nc.tensor.transpose(
                            out=ptr[:, i * 128:(i + 1) * 128],
                            in_=g2[:, (s + i) * 128:(s + i + 1) * 128],
                            identity=identb[:])
                    nc.scalar.copy(
                        out=g3[:, s * 128:(s + nblk) * 128],
                        in_=ptr[:, 0:nblk * 128])
                    s += nblk
                d["g3"] = g3

            def term_stageC_dve(ti, d):
                Wt = W[ti]
                # stage C: into node/rel slotted layout
                slotted = io2.tile([128, Wt], bf16, tag="slotted",
                                   name=f"slotted{ti}", bufs=2)
                so = int(seg_off[ti][kc[ti]])
                nc.gpsimd.local_scatter(
                    slotted[:, 0:so], d["g3"][:], d["sc"][0][:],
                    channels=128, num_elems=so, num_idxs=G2W)
                nc.gpsimd.local_scatter(
                    slotted[:, so:Wt], d["g3"][:], d["sc"][1][:],
                    channels=128, num_elems=Wt - so, num_idxs=G2W)
                # local-side add: broadcast each node's score over its
                # (uniform-width) k-tile segment with a step-0 AP
                lcol = (0, 3)[ti]
                Su = int(S[ti][0])
                sc_b = scols_b[:, lcol:4 * KT:4]
                nc.vector.tensor_tensor(
                    out=slotted[:].rearrange("p (k s) -> p k s", s=Su),
                    in0=slotted[:].rearrange("p (k s) -> p k s", s=Su),
                    in1=sc_b.to_broadcast([128, KT, Su]),
                    op=mybir.AluOpType.add)
                # w = exp(lrelu(e))
                nc.scalar.activation(slotted[:], slotted[:],
                                     mybir.ActivationFunctionType.Lrelu,
                                     alpha=0.01)
                nc.scalar.activation(slotted[:], slotted[:],
                                     mybir.ActivationFunctionType.Exp)
                wb = slotted
                # merge dups via shifted adds masked by the dup grid:
                #   accw[j] = w[j] + N[j+1]*w[j+1]
                #   accw[j] += N[j+1]N[j+2]*accw[j+2]
                # accw/tmp reuse the g3/g2 double-buffers (their routing
                # contents are consumed by this point in the same term)
                accw = io2.tile([128, Wt], bf16, tag="g3", name=f"accw{ti}")
                tmp = io2.tile([128, Wt], bf16, tag="g2", name=f"tmpm{ti}")
                ng_t = d["ng"]
                nc.vector.tensor_tensor(
                    out=tmp[:, 0:Wt - 1], in0=ng_t[:, 1:Wt], in1=wb[:, 1:Wt],
                    op=mybir.AluOpType.mult)
                nc.vector.tensor_tensor(
                    out=accw[:, 0:Wt - 1], in0=wb[:, 0:Wt - 1],
                    in1=tmp[:, 0:Wt - 1], op=mybir.AluOpType.add)
                nc.vector.tensor_copy(out=accw[:, Wt - 1:Wt],
                                      in_=wb[:, Wt - 1:Wt])
                nc.vector.tensor_tensor(
                    out=tmp[:, 0:Wt - 2], in0=ng_t[:, 1:Wt - 1],
                    in1=ng_t[:, 2:Wt], op=mybir.AluOpType.mult)
                nc.vector.tensor_tensor(
                    out=tmp[:, 0:Wt - 2], in0=tmp[:, 0:Wt - 2],
                    in1=accw[:, 2:Wt], op=mybir.AluOpType.mult)
                nc.vector.tensor_tensor(
                    out=accw[:, 0:Wt - 2], in0=accw[:, 0:Wt - 2],
                    in1=tmp[:, 0:Wt - 2], op=mybir.AluOpType.add)
                d["accw"] = accw

            def term_bphase(ti, d, inject=None):
                # B tiles + U^T / z matmuls (stream rel columns)
                accw, scat_t = d["accw"], d["scat"]
                feat_base = (1, 130)[ti]
                psU = aps.tile([128, 1024], f32, tag="psU",
                               name=f"psU{ti}")
                psz = aps.tile([1, 1024], f32, tag="psz",
                               name=f"psz{ti}")
                for jj in range((KT + 1) // 2):
                    k0 = 2 * jj
                    a = int(seg_off[ti][k0])
                    b = int(seg_off[ti][min(k0 + 2, KT)])
                    bt = btp.tile([128, LS_MAX], bf16, tag="bt")
                    nc.gpsimd.local_scatter(
                        bt[:], accw[:, a:b], scat_t[:, a:b],
                        channels=128, num_elems=LS_MAX, num_idxs=b - a)
                    for k in (k0, k0 + 1):
                        if k >= KT:
                            continue
                        off = 1000 * (k & 1)
                        feats = xg[:, k * 262 + feat_base:
                                   k * 262 + feat_base + 128]
                        for n0, n1 in ((0, 512), (512, 1000)):
                            nc.tensor.matmul(
                                psU[:, n0:n1], feats,
                                bt[:, off + n0:off + n1],
                                start=(k == 0), stop=(k == KT - 1))
                            nc.tensor.matmul(
                                psz[0:1, n0:n1], onescol[:, 0:1],
                                bt[:, off + n0:off + n1],
                                start=(k == 0), stop=(k == KT - 1))
                    if inject and jj in inject:
                        inject[jj]()
                nc.vector.tensor_copy(out=res[:, ti * 1000:ti * 1000 + 1000],
                                      in_=psU[:, 0:1000])
                nc.vector.tensor_copy(out=zrow(ti), in_=psz[0:1, 0:1000])

            def term_rs(ti):
                # shard + reduce-scatter this term's [U | z] partials
                # (2 batched DMAs: [p, shard, w] strided into the 8 shards)
                nc.sync.dma_start(
                    out=cc_in[ti].ap().rearrange(
                        "(s x) 1 -> s x", s=NC)[:, 0:128 * RSH].rearrange(
                        "s (p w) -> p s w", p=128),
                    in_=res[:, ti * 1000:(ti + 1) * 1000].rearrange(
                        "p (s w) -> p s w", s=NC))
                nc.sync.dma_start(
                    out=cc_in[ti].ap().rearrange(
                        "(s x) 1 -> s x", s=NC)[:, 128 * RSH:SHW].rearrange(
                        "s (o w) -> o s w", o=1),
                    in_=zrow(ti).rearrange("o (s w) -> o s w", s=NC))
                nc.gpsimd.collective_compute(
                    "ReduceScatter", mybir.AluOpType.add,
                    replica_groups=groups,
                    ins=[cc_in[ti].ap()], outs=[cc_out[ti].ap()])

            red = pp_.tile([128, 2 * RSH], f32, tag="red")
            zt = pp_.tile([128, 2 * RSH], f32, tag="zt")

            def term_fin(ti):
                # load this term's reduced shard and divide by z
                nc.sync.dma_start(
                    out=red[:, ti * RSH:(ti + 1) * RSH],
                    in_=cc_out[ti].ap()[0:128 * RSH, :].rearrange(
                        "(p w) 1 -> p w", p=128))
                nc.sync.dma_start(
                    out=zt[:, ti * RSH:(ti + 1) * RSH],
                    in_=cc_out[ti].ap()[128 * RSH:, :].rearrange(
                        "(o w) 1 -> o w", o=1).to_broadcast([128, RSH]))
                sl = slice(ti * RSH, (ti + 1) * RSH)
                nc.vector.tensor_scalar_add(zt[:, sl], zt[:, sl], EPS)
                nc.vector.reciprocal(zt[:, sl], zt[:, sl])
                nc.vector.tensor_tensor(out=red[:, sl], in0=red[:, sl],
                                        in1=zt[:, sl],
                                        op=mybir.AluOpType.mult)

            d0 = term_tables_pre(0)
            d1 = term_tables_pre(1)
            # tslice4's trigger waits on the AllGather and would
            # head-of-line block later table loads, so the big post-tables
            # (only needed from stage C on) are issued after it
            tslice4 = pp_.tile([128, TS4], bf16, tag="tslice4")
            nc.sync.dma_start(
                out=tslice4[:],
                in_=stab.ap().rearrange("(p w) 1 -> p w", p=128))
            term_tables_post(0, d0)
            term_tables_post(1, d1)
            term_route_head(0, d0)
            term_route_head(1, d1)
            term_stageA_T(0, d0)
            term_stageA_T(1, d1)
            term_stageC_dve(0, d0)
            term_stageC_dve(1, d1)
            term_bphase(0, d0)
            term_bphase(1, d1, inject={
                2: lambda: term_rs(0),
                16: lambda: term_fin(0),
            })
            term_rs(1)

            # ---- phase 4: finalize own 125-rel shard ----
            term_fin(1)
            nc.vector.tensor_tensor(out=red[:, 0:RSH], in0=red[:, 0:RSH],
                                    in1=red[:, RSH:2 * RSH],
                                    op=mybir.AluOpType.add)
            # transpose [128 feats, 125 rels] -> out [125, 128]
            ident = pp_.tile([128, 128], f32, tag="ident")
            make_identity(nc, ident[:])
            otile = pp_.tile([128, 128], f32, tag="otile")
            pst = aps.tile([128, 1024], f32, tag="psU", name="pst")
            nc.tensor.transpose(
                out=pst[:RSH, 0:128], in_=red[:, 0:RSH],
                identity=ident[:])
            nc.vector.tensor_copy(out=otile[:RSH, :], in_=pst[:RSH, 0:128])
            nc.sync.dma_start(out=out_d.ap()[:, :], in_=otile[:RSH, :])

    nc.compile()
    return nc


_CACHE = {}


def kernel(x_e, edge_index, rel, W_h, W_t, a_h1, a_h2, a_t1, a_t2):
    import ml_dtypes  # noqa: F401  (bfloat16 dtype registration)
    from concourse.bass_utils import run_bass_kernel_spmd

    x_e = np.asarray(x_e, dtype=np.float32)
    ins, dims = _host_prep(edge_index, rel)
    Wcat = _host_weights(
        np.asarray(W_h, np.float32), np.asarray(W_t, np.float32),
        np.asarray(a_h1, np.float32), np.asarray(a_h2, np.float32),
        np.asarray(a_t1, np.float32), np.asarray(a_t2, np.float32))

    key = (tuple(dims["W"]), dims["Wc"], dims["Gmax"], tuple(dims["kc"]),
           tuple(map(tuple, dims["S"])), dims["W2"], dims["SEG"],
           dims["NPASS"], "v2")
    if key not in _CACHE:
        _CACHE[key] = _build_bass(dims)
    nc = _CACHE[key]

    wct_b = Wcat.astype("bfloat16")
    in_maps = []
    for c in range(NC):
        xs = np.zeros((NSH, EH), dtype=np.float32)
        xs[:SH] = x_e[c * SH:(c + 1) * SH]
        m = {"xT": np.ascontiguousarray(xs.T).astype("bfloat16"),
             "Wct": wct_b}
        for ti in range(2):
            for nm in ("runstart", "ngrid", "scat", "fills"):
                m[f"{nm}{ti}"] = ins[c][f"{nm}{ti}"]
            for j in (1, 2):
                m[f"sa{j}_{ti}"] = ins[c][f"sa{j}_{ti}"]
                m[f"sc{j}_{ti}"] = ins[c][f"sc{j}_{ti}"]
        in_maps.append(m)

    trace = bool(int(os.environ.get("GAT_TRACE", "0")))
    res = run_bass_kernel_spmd(nc, in_maps, core_ids=list(range(NC)),
                               trace=trace)
    kernel.last_exec_time_ns = res.exec_time_ns
    kernel.last_results = res
    out = np.concatenate(
        [np.asarray(res.results[c]["out"], dtype=np.float32)
         for c in range(NC)], axis=0)
    return out


kernel.last_exec_time_ns = None
